# revision 16
# baseline (speedup 1.0000x reference)
"""FCOS head kernel for Trainium2 (8 NeuronCores, data-parallel over batch).

Per image (one per core): 5 FPN levels, each through two 4-layer conv
towers (3x3 conv 256->256 + GroupNorm(32) + ReLU), then head convs:
logits (80ch) + centerness (1ch) from the cls tower, exp(scale*reg)
(4ch) from the box tower.  Output: concat([logits, reg, ctr]) per level
flattened and concatenated, plus the (input-independent) FCOS grid
locations.

Implementation: conv3x3 = 9 shifted accumulating matmuls per
(cin-block, cout-block) on the TensorEngine in float32r (full fp32
storage, ~bf16-rate streaming).  Activations live in DRAM in a
row-padded layout ([2, 128, H+2, W+2], zero borders) so a conv chunk is
one contiguous DMA; GroupNorm is two-phase (bn_stats per chunk ->
cross-partition group reduce via tiny mask matmuls -> fused
relu(a*x+b) apply on load of the next conv).
"""

import sys

sys.path.insert(0, "/opt/trn_rl_repo")

import numpy as np

import concourse.bass as bass
import concourse.tile as tile
import concourse.mybir as mybir
from concourse import bacc
from concourse.bass_utils import run_bass_kernel_spmd

F32 = mybir.dt.float32
F32R = mybir.dt.float32r
AF = mybir.ActivationFunctionType

N_CORES = 8
C = 256
NCONV = 4
GROUPS = 32
EPS = 1e-5
STRIDES = (8, 16, 32, 64, 128)

LEVELS = [(100, 152), (50, 76), (25, 38), (13, 19), (7, 10)]
ROWPLANS = [
    [3] * 32 + [2] * 2,
    [5] * 10,
    [9, 9, 7],
    [13],
    [7],
]


def _chunks(rowplan):
    """[(r0, R)] for each chunk."""
    out = []
    r0 = 0
    for r in rowplan:
        out.append((r0, r))
        r0 += r
    return out


def build_nc(levels=None, rowplans=None, nconv=NCONV, nrep=1):
    levels = levels or LEVELS
    rowplans = rowplans or ROWPLANS
    nlev = len(levels)

    nc = bacc.Bacc("TRN2", target_bir_lowering=False, debug=False)

    # ---------------- DRAM declarations ----------------
    fpad = [
        nc.dram_tensor(f"fpad{l}", [2, 128, h + 2, w + 2], F32R, kind="ExternalInput")
        for l, (h, w) in enumerate(levels)
    ]
    wts = nc.dram_tensor("wts", [128, NCONV, 2, 3, 3, 2, 2, 128], F32R,
                         kind="ExternalInput")
    wh_ct = nc.dram_tensor("wh_ct", [128, 3, 3, 2, 81], F32R, kind="ExternalInput")
    wh_bt = nc.dram_tensor("wh_bt", [128, 3, 3, 2, 4], F32R, kind="ExternalInput")
    gnp = nc.dram_tensor("gnp", [128, NCONV, 2, 2, 2], F32, kind="ExternalInput")
    cbias = nc.dram_tensor("cbias", [128, NCONV, 2, 2], F32, kind="ExternalInput")
    hb = nc.dram_tensor("hb", [128, 1], F32, kind="ExternalInput")
    hexp_s = nc.dram_tensor("hexp_s", [128, nlev], F32, kind="ExternalInput")
    hexp_b = nc.dram_tensor("hexp_b", [128, nlev], F32, kind="ExternalInput")
    gmask = nc.dram_tensor("gmask", [128, 16], F32, kind="ExternalInput")
    emask = nc.dram_tensor("emask", [128, 128], F32, kind="ExternalInput")

    outs = [
        nc.dram_tensor(f"out{l}", [85, h, w], F32, kind="ExternalOutput")
        for l, (h, w) in enumerate(levels)
    ]

    # raw conv-output scratch, padded layout; [tower][level][parity]
    raw = [
        [
            [
                nc.dram_tensor(f"raw_{t}_{l}_{p}", [2, 128, h + 2, w + 2], F32R)
                for p in range(2)
            ]
            for l, (h, w) in enumerate(levels)
        ]
        for t in range(2)
    ]

    stats_handles = {}
    ab_handles = {}

    with tile.TileContext(nc) as tc:
        with (
            tc.tile_pool(name="consts", bufs=1) as consts,
            tc.tile_pool(name="wpool", bufs=2) as wpool,
            tc.tile_pool(name="inpool", bufs=4) as inpool,
            tc.tile_pool(name="outpool", bufs=4) as outpool,
            tc.tile_pool(name="hdpool", bufs=3) as hdpool,
            tc.tile_pool(name="statspool", bufs=12) as statspool,
            tc.tile_pool(name="smallpool", bufs=8) as smallpool,
            tc.tile_pool(name="abpool", bufs=6) as abpool,
            tc.tile_pool(name="psmain", bufs=6, space="PSUM") as psmain,
            tc.tile_pool(name="psgn", bufs=1, space="PSUM") as psgn,
        ):
            # ---------------- constants in SBUF ----------------
            gmask_sb = consts.tile([128, 16], F32, tag="gmask")
            nc.sync.dma_start(gmask_sb[:], gmask[:])
            emask_sb = consts.tile([128, 128], F32, tag="emask")
            nc.sync.dma_start(emask_sb[:], emask[:])
            gnp_sb = consts.tile([128, NCONV, 2, 2, 2], F32, tag="gnp")
            nc.sync.dma_start(gnp_sb[:], gnp[:])
            cbias_sb = consts.tile([128, NCONV, 2, 2], F32, tag="cbias")
            nc.sync.dma_start(cbias_sb[:], cbias[:])
            hb_sb = consts.tile([128, 1], F32, tag="hb")
            nc.sync.dma_start(hb_sb[:], hb[:])
            hexp_s_sb = consts.tile([128, nlev], F32, tag="hexp_s")
            nc.sync.dma_start(hexp_s_sb[:], hexp_s[:])
            hexp_b_sb = consts.tile([128, nlev], F32, tag="hexp_b")
            nc.sync.dma_start(hexp_b_sb[:], hexp_b[:])
            wh_ct_sb = consts.tile([128, 3, 3, 2, 81], F32R, tag="wh_ct")
            nc.sync.dma_start(wh_ct_sb[:], wh_ct[:])
            wh_bt_sb = consts.tile([128, 3, 3, 2, 4], F32R, tag="wh_bt")
            nc.sync.dma_start(wh_bt_sb[:], wh_bt[:])
            epsc = consts.tile([128, 1], F32, tag="epsc")
            nc.vector.memset(epsc[:], EPS)

            # zero tile for clearing raw borders
            zmax = max(max(4 * (w + 2), 4 * h) for (h, w) in levels)
            zt = consts.tile([128, zmax], F32, tag="zt")
            nc.vector.memset(zt[:], 0.0)

            # ---------------- zero raw borders ----------------
            for t in range(2):
                for l, (h, w) in enumerate(levels):
                    wp = w + 2
                    for p in range(2):
                        r_ap = raw[t][l][p].ap().rearrange("b c h w -> c b h w")
                        r_flat = raw[t][l][p].ap().rearrange(
                            "b c h w -> c b (h w)"
                        )
                        for b in range(2):
                            # top and bottom rows
                            nc.sync.dma_start(
                                r_ap[:, b, :: h + 1, :],
                                zt[:, : 2 * wp].rearrange(
                                    "c (r w) -> c r w", r=2
                                ).bitcast(F32R),
                            )
                            # side cols via adjacent (r, w+1),(r+1, 0) pairs
                            nc.sync.dma_start(
                                r_flat[:, b, wp + w + 1 : wp + w + 1 + h * wp]
                                .rearrange("c (r w) -> c r w", w=wp)[:, :, 0:2],
                                zt[:, : 2 * h].rearrange(
                                    "c (r w) -> c r w", r=h
                                ).bitcast(F32R),
                            )
                            # col 0 of row 1
                            nc.sync.dma_start(
                                r_flat[:, b, wp : wp + 1], zt[:, :1].bitcast(F32R)
                            )

            # ---------------- GN tail ----------------
            def emit_tail(t, i, l):
                """Compute per-channel (a, b) for GN+ReLU of conv i of tower t
                at level l, from saved bn stats.  Returns the ab tile."""
                key = (t, i, l)
                st = stats_handles.pop(key)
                nch = len(rowplans[l])
                mv = smallpool.tile([128, 2, 2], F32, tag="mv")
                nc.vector.bn_aggr(mv[:, 0, :], st[:, 0, :nch, :])
                nc.vector.bn_aggr(mv[:, 1, :], st[:, 1, :nch, :])
                r4 = smallpool.tile([128, 4], F32, tag="r4")
                nc.vector.tensor_copy(r4[:, 0:2], mv[:, :, 0])
                nc.vector.tensor_mul(r4[:, 2:4], mv[:, :, 0], mv[:, :, 0])
                nc.vector.tensor_add(r4[:, 2:4], r4[:, 2:4], mv[:, :, 1])
                gs = psgn.tile([128, 4], F32, tag="gs")
                nc.tensor.matmul(gs[0:16, :], gmask_sb[:], r4[:], start=True,
                                 stop=True)
                si = smallpool.tile([128, 4], F32, tag="si")
                nc.vector.memset(si[:], 0.0)
                nc.vector.tensor_copy(si[0:16, :], gs[0:16, :])
                tmp = smallpool.tile([128, 2], F32, tag="tmp")
                nc.vector.tensor_mul(tmp[0:16, :], si[0:16, 0:2], si[0:16, 0:2])
                nc.vector.tensor_tensor(si[0:16, 2:4], si[0:16, 2:4], tmp[0:16, :],
                                        mybir.AluOpType.subtract)
                nc.scalar.activation(si[0:16, 2:4], si[0:16, 2:4], AF.Sqrt,
                                     bias=epsc[0:16, :], scale=1.0)
                nc.vector.reciprocal(si[0:16, 2:4], si[0:16, 2:4])
                bc = psgn.tile([128, 4], F32, tag="bc")
                nc.tensor.matmul(bc[:], emask_sb[:], si[:], start=True, stop=True)
                ab = abpool.tile([128, 2, 2], F32, tag="ab")
                nc.vector.tensor_mul(ab[:, :, 0], gnp_sb[:, i, t, :, 0], bc[:, 2:4])
                tmp2 = smallpool.tile([128, 2], F32, tag="tmp2")
                nc.vector.tensor_mul(tmp2[:], bc[:, 0:2], ab[:, :, 0])
                nc.vector.tensor_tensor(ab[:, :, 1], gnp_sb[:, i, t, :, 1], tmp2[:],
                                        mybir.AluOpType.subtract)
                return ab

            def get_ab(t, i, l):
                key = (t, i, l)
                if key not in ab_handles:
                    ab_handles[key] = emit_tail(t, i, l)
                return ab_handles[key]

            # ---------------- conv in-chunk load + apply ----------------
            def load_chunk(src_ap_r, l, r0, R, ab):
                """DMA rows [r0, r0+R+2) of padded src into an in-tile (data at
                offset 1), then optionally apply relu(a*x+b) on the interior
                (real image rows/cols only -- padded borders must stay 0)."""
                h, w = levels[l]
                wp = w + 2
                smax = (max(rowplans[l]) + 2) * wp + 3
                s_used = (R + 2) * wp
                in_t = inpool.tile([128, 2, smax], F32R, tag=f"in{l}")
                nc.sync.dma_start(
                    in_t[:, :, 0:1],
                    zt[:, :2].rearrange("c (b g) -> c b g", b=2).bitcast(F32R),
                )
                nc.sync.dma_start(
                    in_t[:, :, 1 + s_used : 3 + s_used],
                    zt[:, :4].rearrange("c (b g) -> c b g", b=2).bitcast(F32R),
                )
                nc.sync.dma_start(
                    in_t[:, :, 1 : 1 + s_used].rearrange(
                        "c b (r w) -> c b r w", w=wp
                    ),
                    src_ap_r[:, :, r0 : r0 + R + 2, :],
                )
                if ab is not None:
                    k0 = 1 if r0 == 0 else 0
                    k1 = R + 1 if r0 + R == h else R + 2
                    for cb in range(2):
                        view = (
                            in_t[:, cb, 1 + k0 * wp : 1 + k1 * wp]
                            .rearrange("c (r w) -> c r w", w=wp)[:, :, 1 : w + 1]
                        )
                        nc.scalar.activation(
                            view, view, AF.Relu,
                            bias=ab[:, cb, 1:2], scale=ab[:, cb, 0:1],
                        )
                return in_t

            # ---------------- one tower-conv job ----------------
            def conv_job(t, i, l, w_t):
                h, w = levels[l]
                wp = w + 2
                plan = _chunks(rowplans[l])
                nch = len(plan)
                src = fpad[l].ap() if i == 0 else raw[t][l][(i - 1) % 2].ap()
                src_r = src.rearrange("b c h w -> c b h w")
                dst_r = raw[t][l][i % 2].ap().rearrange("b c h w -> c b h w")
                ab = None if i == 0 else get_ab(t, i - 1, l)
                st = statspool.tile([128, 2, 34, 6], F32, tag="stats")
                stats_handles[(t, i, l)] = st
                rwmax = max(r for _, r in plan) * w
                for ci, (r0, R) in enumerate(plan):
                    in_t = load_chunk(src_r, l, r0, R, ab)
                    n = R * wp
                    n_mm = n + (n & 1)
                    out_sb = outpool.tile([128, 2, rwmax], F32, tag="outsb")
                    for ob in range(2):
                        ps = psmain.tile([128, 512], F32, tag="ps")
                        first = True
                        for dy in range(3):
                            for dx in range(3):
                                for cb in range(2):
                                    o = dy * wp + dx
                                    nc.tensor.matmul(
                                        ps[:, :n_mm],
                                        w_t[:, dy, dx, cb, ob, :],
                                        in_t[:, cb, o : o + n_mm],
                                        start=first,
                                        stop=(dy == 2 and dx == 2 and cb == 1),
                                    )
                                    first = False
                        # copy interior + conv bias
                        nc.scalar.activation(
                            out_sb[:, ob, : R * w].rearrange(
                                "c (r w) -> c r w", w=w
                            ),
                            ps[:, :n].rearrange("c (r w) -> c r w", w=wp)[
                                :, :, 1 : w + 1
                            ],
                            AF.Identity,
                            bias=cbias_sb[:, i, t, ob : ob + 1],
                            scale=1.0,
                        )
                        nc.vector.bn_stats(st[:, ob, ci, :], out_sb[:, ob, : R * w])
                    for ob in range(2):
                        nc.sync.dma_start(
                            dst_r[:, ob, 1 + r0 : 1 + r0 + R, 1 : w + 1],
                            out_sb[:, ob, : R * w].rearrange(
                                "c (r w) -> c r w", w=w
                            ).bitcast(F32R),
                        )

            # ---------------- one head job ----------------
            def head_job(l):
                h, w = levels[l]
                wp = w + 2
                plan = _chunks(rowplans[l])
                ab_ct = get_ab(0, nconv - 1, l)
                ab_bt = get_ab(1, nconv - 1, l)
                ct_r = raw[0][l][(nconv - 1) % 2].ap().rearrange("b c h w -> c b h w")
                bt_r = raw[1][l][(nconv - 1) % 2].ap().rearrange("b c h w -> c b h w")
                out_ap = outs[l].ap()
                rwmax = max(r for _, r in plan) * w
                for ci, (r0, R) in enumerate(plan):
                    in_ct = load_chunk(ct_r, l, r0, R, ab_ct)
                    in_bt = load_chunk(bt_r, l, r0, R, ab_bt)
                    n = R * wp
                    n_mm = n + (n & 1)
                    ps_ct = psmain.tile([128, 512], F32, tag="ps")
                    ps_bt = psmain.tile([128, 512], F32, tag="ps")
                    for which, (ps, in_t, w_sb, m) in enumerate(
                        [(ps_ct, in_ct, wh_ct_sb, 81), (ps_bt, in_bt, wh_bt_sb, 4)]
                    ):
                        first = True
                        for dy in range(3):
                            for dx in range(3):
                                for cb in range(2):
                                    o = dy * wp + dx
                                    nc.tensor.matmul(
                                        ps[0:m, :n_mm],
                                        w_sb[:, dy, dx, cb, :],
                                        in_t[:, cb, o : o + n_mm],
                                        start=first,
                                        stop=(dy == 2 and dx == 2 and cb == 1),
                                    )
                                    first = False
                    hd1 = hdpool.tile([128, rwmax], F32, tag="hd1")
                    hd2 = hdpool.tile([128, rwmax], F32, tag="hd2")
                    nc.scalar.activation(
                        hd1[0:81, : R * w].rearrange("c (r w) -> c r w", w=w),
                        ps_ct[0:81, :n].rearrange("c (r w) -> c r w", w=wp)[
                            :, :, 1 : w + 1
                        ],
                        AF.Identity,
                        bias=hb_sb[0:81, :],
                        scale=1.0,
                    )
                    nc.scalar.activation(
                        hd2[0:4, : R * w].rearrange("c (r w) -> c r w", w=w),
                        ps_bt[0:4, :n].rearrange("c (r w) -> c r w", w=wp)[
                            :, :, 1 : w + 1
                        ],
                        AF.Exp,
                        bias=hexp_b_sb[0:4, l : l + 1],
                        scale=hexp_s_sb[0:4, l : l + 1],
                    )
                    nc.sync.dma_start(
                        out_ap[0:80, r0 : r0 + R, :], hd1[0:80, : R * w]
                    )
                    nc.sync.dma_start(
                        out_ap[84:85, r0 : r0 + R, :], hd1[80:81, : R * w]
                    )
                    nc.sync.dma_start(
                        out_ap[80:84, r0 : r0 + R, :], hd2[0:4, : R * w]
                    )

            # ---------------- emission: layer-major waves ----------------
            def emit_all():
                for i in range(nconv):
                    for t in range(2):
                        w_t = wpool.tile([128, 3, 3, 2, 2, 128], F32R, tag="wt")
                        nc.sync.dma_start(w_t[:], wts[:, i, t])
                        for l in range(nlev):
                            conv_job(t, i, l, w_t)
                for l in range(nlev):
                    head_job(l)

            if nrep == 1:
                emit_all()
            else:
                with tc.For_i(0, nrep, 1):
                    emit_all()

    nc.compile()
    return nc


# ---------------- host side ----------------

_NC_CACHE = {}


def _get_nc():
    if "nc" not in _NC_CACHE:
        _NC_CACHE["nc"] = build_nc()
    return _NC_CACHE["nc"]


def _prep_common(cls_w, cls_b, cls_gn_g, cls_gn_b, box_w, box_b, box_gn_g,
                 box_gn_b, logits_w, logits_b, ctr_w, ctr_b, reg_w, reg_b,
                 scales, nlev):
    f32 = np.float32
    # tower weights: [ci, conv, T, dy, dx, cb, ob, co]
    wt = np.stack([cls_w, box_w], axis=1).astype(f32)  # [4, 2, 256o, 256i, 3, 3]
    wt = wt.reshape(NCONV, 2, 2, 128, 2, 128, 3, 3)  # [i,T,ob,co,cb,ci,dy,dx]
    wts = np.ascontiguousarray(wt.transpose(5, 0, 1, 6, 7, 4, 2, 3))
    # head weights
    hw_ct = np.concatenate([logits_w, ctr_w], axis=0).astype(f32)  # [81,256,3,3]
    hw_ct = hw_ct.reshape(81, 2, 128, 3, 3)
    wh_ct = np.ascontiguousarray(hw_ct.transpose(2, 3, 4, 1, 0))  # [ci,dy,dx,cb,81]
    hw_bt = reg_w.astype(f32).reshape(4, 2, 128, 3, 3)
    wh_bt = np.ascontiguousarray(hw_bt.transpose(2, 3, 4, 1, 0))
    # GN params [p, conv, T, blk, {g,b}]
    gg = np.stack([cls_gn_g, box_gn_g], axis=1).reshape(NCONV, 2, 2, 128)
    gb = np.stack([cls_gn_b, box_gn_b], axis=1).reshape(NCONV, 2, 2, 128)
    gnp = np.ascontiguousarray(
        np.stack([gg, gb], axis=-1).transpose(3, 0, 1, 2, 4)
    ).astype(f32)
    cb_ = np.stack([cls_b, box_b], axis=1).reshape(NCONV, 2, 2, 128)
    cbias = np.ascontiguousarray(cb_.transpose(3, 0, 1, 2)).astype(f32)
    hb = np.zeros((128, 1), f32)
    hb[0:80, 0] = logits_b
    hb[80, 0] = ctr_b[0]
    hexp_s = np.zeros((128, nlev), f32)
    hexp_b = np.zeros((128, nlev), f32)
    for l in range(nlev):
        hexp_s[0:4, l] = scales[l]
        hexp_b[0:4, l] = scales[l] * reg_b
    gmask = np.zeros((128, 16), f32)
    for p in range(128):
        gmask[p, p // 8] = 1.0 / 8.0
    emask = np.zeros((128, 128), f32)
    for c_ in range(128):
        emask[c_ // 8, c_] = 1.0
    return dict(wts=wts, wh_ct=wh_ct, wh_bt=wh_bt, gnp=gnp, cbias=cbias,
                hb=hb, hexp_s=hexp_s, hexp_b=hexp_b, gmask=gmask, emask=emask)


def _locations(levels, strides):
    locs = []
    for l, (h, w) in enumerate(levels):
        s = strides[l]
        sx = np.arange(w, dtype=np.float32) * s + s // 2
        sy = np.arange(h, dtype=np.float32) * s + s // 2
        yy, xx = np.meshgrid(sy, sx, indexing="ij")
        locs.append(np.stack([xx.reshape(-1), yy.reshape(-1)], axis=1))
    return np.concatenate(locs, axis=0).astype(np.float32)


def _make_in_maps(inputs):
    feats = [np.asarray(inputs[f"feat{l}"], np.float32) for l in range(len(LEVELS))]
    B = feats[0].shape[0]
    assert B == N_CORES
    nlev = len(LEVELS)
    common = _prep_common(
        np.asarray(inputs["cls_w"]), np.asarray(inputs["cls_b"]),
        np.asarray(inputs["cls_gn_g"]), np.asarray(inputs["cls_gn_b"]),
        np.asarray(inputs["box_w"]), np.asarray(inputs["box_b"]),
        np.asarray(inputs["box_gn_g"]), np.asarray(inputs["box_gn_b"]),
        np.asarray(inputs["logits_w"]), np.asarray(inputs["logits_b"]),
        np.asarray(inputs["ctr_w"]), np.asarray(inputs["ctr_b"]),
        np.asarray(inputs["reg_w"]), np.asarray(inputs["reg_b"]),
        np.asarray(inputs["scales"]), nlev)
    in_maps = []
    for b in range(B):
        m = dict(common)
        for l, (h, w) in enumerate(LEVELS):
            f = feats[l][b].reshape(2, 128, h, w)
            m[f"fpad{l}"] = np.pad(
                f, ((0, 0), (0, 0), (1, 1), (1, 1))
            ).astype(np.float32)
        in_maps.append(m)
    return in_maps


def kernel(feat0, feat1, feat2, feat3, feat4, cls_w, cls_b, cls_gn_g, cls_gn_b,
           box_w, box_b, box_gn_g, box_gn_b, logits_w, logits_b, ctr_w, ctr_b,
           reg_w, reg_b, scales):
    in_maps = _make_in_maps(dict(
        feat0=feat0, feat1=feat1, feat2=feat2, feat3=feat3, feat4=feat4,
        cls_w=cls_w, cls_b=cls_b, cls_gn_g=cls_gn_g, cls_gn_b=cls_gn_b,
        box_w=box_w, box_b=box_b, box_gn_g=box_gn_g, box_gn_b=box_gn_b,
        logits_w=logits_w, logits_b=logits_b, ctr_w=ctr_w, ctr_b=ctr_b,
        reg_w=reg_w, reg_b=reg_b, scales=scales))
    B = N_CORES
    nlev = len(LEVELS)

    nc = _get_nc()
    res = run_bass_kernel_spmd(nc, in_maps, list(range(N_CORES)))

    out_rows = []
    for b in range(B):
        parts = [res.results[b][f"out{l}"].reshape(-1) for l in range(nlev)]
        out_rows.append(np.concatenate(parts))
    out = np.stack(out_rows).astype(np.float32)
    locs = _locations(LEVELS, STRIDES)
    return out, locs


if __name__ == "__main__":
    import time

    t0 = time.time()
    nc = build_nc()
    print(f"build_nc: {time.time() - t0:.1f}s")


# revision 27
# speedup vs baseline: 1.1357x; 1.1357x over previous
"""FCOS head kernel for Trainium2 (8 NeuronCores, data-parallel over batch).

Per image (one per core): 5 FPN levels, each through two 4-layer conv
towers (3x3 conv 256->256 + GroupNorm(32) + ReLU), then head convs:
logits (80ch) + centerness (1ch) from the cls tower, exp(scale*reg)
(4ch) from the box tower.  Output: concat([logits, reg, ctr]) per level
flattened and concatenated, plus the (input-independent) FCOS grid
locations.

Implementation: conv3x3 = 9 shifted accumulating matmuls per
(cin-block, cout-block) on the TensorEngine in float32r (full fp32
storage, ~bf16-rate streaming).  Activations live in DRAM in a
row-padded layout ([2, 128, H+2, W+2], zero borders) so a conv chunk is
one contiguous DMA; GroupNorm is two-phase (bn_stats per chunk ->
cross-partition group reduce via tiny mask matmuls -> fused
relu(a*x+b) apply on load of the next conv).
"""

import sys

sys.path.insert(0, "/opt/trn_rl_repo")

import numpy as np

import concourse.bass as bass
import concourse.tile as tile
import concourse.mybir as mybir
from concourse import bacc
from concourse.bass_utils import run_bass_kernel_spmd

F32 = mybir.dt.float32
F32R = mybir.dt.float32r
F16 = mybir.dt.float16
AF = mybir.ActivationFunctionType

N_CORES = 8
C = 256
NCONV = 4
GROUPS = 32
EPS = 1e-5
STRIDES = (8, 16, 32, 64, 128)

ADT = mybir.dt.float16  # activation/weight dtype for the conv datapath

LEVELS = [(100, 152), (50, 76), (25, 38), (13, 19), (7, 10)]
# Per level: list of strips; each strip is a list of chunk row-counts.
# One strip = one SBUF window load; chunks within share its halo.
STRIPPLANS = [
    [[3, 3, 3, 3, 3]] * 6 + [[3, 3, 2, 2]],
    [[5, 5, 5, 5, 5]] * 2,
    [[9, 9, 7]],
    [[13]],
    [[7]],
]
ROWPLANS = [sum(sp, []) for sp in STRIPPLANS]


def _chunks(rowplan):
    """[(r0, R)] for each chunk."""
    out = []
    r0 = 0
    for r in rowplan:
        out.append((r0, r))
        r0 += r
    return out


def build_nc(levels=None, stripplans=None, nconv=NCONV, nrep=1, adt=None):
    levels = levels or LEVELS
    stripplans = stripplans or STRIPPLANS
    rowplans = [sum(sp, []) for sp in stripplans]
    nlev = len(levels)
    adt = adt or ADT

    nc = bacc.Bacc("TRN2", target_bir_lowering=False, debug=False)

    # ---------------- DRAM declarations ----------------
    fpad = [
        nc.dram_tensor(f"fpad{l}", [2, 128, h + 2, w + 2], adt, kind="ExternalInput")
        for l, (h, w) in enumerate(levels)
    ]
    wts = nc.dram_tensor("wts", [128, NCONV, 2, 3, 3, 2, 2, 128], adt,
                         kind="ExternalInput")
    wh_ct = nc.dram_tensor("wh_ct", [128, 3, 3, 2, 81], adt, kind="ExternalInput")
    wh_bt = nc.dram_tensor("wh_bt", [128, 3, 3, 2, 4], adt, kind="ExternalInput")
    gnp = nc.dram_tensor("gnp", [128, NCONV, 2, 2, 2], F32, kind="ExternalInput")
    cbias = nc.dram_tensor("cbias", [128, NCONV, 2, 2], F32, kind="ExternalInput")
    hb = nc.dram_tensor("hb", [128, 1], F32, kind="ExternalInput")
    hexp_s = nc.dram_tensor("hexp_s", [128, nlev], F32, kind="ExternalInput")
    hexp_b = nc.dram_tensor("hexp_b", [128, nlev], F32, kind="ExternalInput")
    gmask = nc.dram_tensor("gmask", [128, nlev, 16], F32, kind="ExternalInput")
    emask = nc.dram_tensor("emask", [128, 128], F32, kind="ExternalInput")

    outs = [
        nc.dram_tensor(f"out{l}", [85, h, w], F32, kind="ExternalOutput")
        for l, (h, w) in enumerate(levels)
    ]

    # raw conv-output scratch, padded layout; [tower][level][parity]
    raw = [
        [
            [
                nc.dram_tensor(f"raw_{t}_{l}_{p}", [2, 128, h + 2, w + 2], adt)
                for p in range(2)
            ]
            for l, (h, w) in enumerate(levels)
        ]
        for t in range(2)
    ]

    stats_handles = {}
    ab_handles = {}

    with tile.TileContext(nc) as tc:
        with (
            tc.tile_pool(name="consts", bufs=1) as consts,
            tc.tile_pool(name="wpool", bufs=2) as wpool,
            tc.tile_pool(name="inpool", bufs=4) as inpool,
            tc.tile_pool(name="outpool", bufs=4) as outpool,
            tc.tile_pool(name="hdpool", bufs=3) as hdpool,
            tc.tile_pool(name="statspool", bufs=12) as statspool,
            tc.tile_pool(name="smallpool", bufs=8) as smallpool,
            tc.tile_pool(name="abpool", bufs=6) as abpool,
            tc.tile_pool(name="psmain", bufs=6, space="PSUM") as psmain,
            tc.tile_pool(name="psgn", bufs=1, space="PSUM") as psgn,
        ):
            # ---------------- constants in SBUF ----------------
            gmask_sb = consts.tile([128, nlev, 16], F32, tag="gmask")
            nc.sync.dma_start(gmask_sb[:], gmask[:])
            emask_sb = consts.tile([128, 128], F32, tag="emask")
            nc.sync.dma_start(emask_sb[:], emask[:])
            gnp_sb = consts.tile([128, NCONV, 2, 2, 2], F32, tag="gnp")
            nc.sync.dma_start(gnp_sb[:], gnp[:])
            cbias_sb = consts.tile([128, NCONV, 2, 2], F32, tag="cbias")
            nc.sync.dma_start(cbias_sb[:], cbias[:])
            hb_sb = consts.tile([128, 1], F32, tag="hb")
            nc.sync.dma_start(hb_sb[:], hb[:])
            hexp_s_sb = consts.tile([128, nlev], F32, tag="hexp_s")
            nc.sync.dma_start(hexp_s_sb[:], hexp_s[:])
            hexp_b_sb = consts.tile([128, nlev], F32, tag="hexp_b")
            nc.sync.dma_start(hexp_b_sb[:], hexp_b[:])
            wh_ct_sb = consts.tile([128, 3, 3, 2, 81], adt, tag="wh_ct")
            nc.sync.dma_start(wh_ct_sb[:], wh_ct[:])
            wh_bt_sb = consts.tile([128, 3, 3, 2, 4], adt, tag="wh_bt")
            nc.sync.dma_start(wh_bt_sb[:], wh_bt[:])
            epsc = consts.tile([128, 1], F32, tag="epsc")
            nc.vector.memset(epsc[:], EPS)

            # zero tile for clearing raw borders
            zmax = max(max(4 * (w + 2), 4 * h) for (h, w) in levels)
            zt = consts.tile([128, zmax], F32, tag="zt")
            nc.vector.memset(zt[:], 0.0)
            zta = zt.bitcast(adt) if adt == F32R else consts.tile(
                [128, zmax], adt, tag="zta")
            if adt != F32R:
                nc.gpsimd.memset(zta[:], 0.0)

            # ---------------- zero raw borders ----------------
            for t in range(2):
                for l, (h, w) in enumerate(levels):
                    wp = w + 2
                    for p in range(2):
                        r_ap = raw[t][l][p].ap().rearrange("b c h w -> c b h w")
                        for b in range(2):
                            # top and bottom padded rows (side borders are
                            # written as part of every full-width row store)
                            nc.sync.dma_start(
                                r_ap[:, b, :: h + 1, :],
                                zta[:, : 2 * wp].rearrange(
                                    "c (r w) -> c r w", r=2
                                ),
                            )

            # ---------------- GN tail ----------------
            def emit_tail(t, i, l):
                """Compute per-channel (a, b) for GN+ReLU of conv i of tower t
                at level l, from saved bn stats.  Returns the ab tile."""
                key = (t, i, l)
                st = stats_handles.pop(key)
                nch = len(rowplans[l])
                mv = smallpool.tile([128, 2, 2], F32, tag="mv")
                nc.vector.bn_aggr(mv[:, 0, :], st[:, 0, :nch, :])
                nc.vector.bn_aggr(mv[:, 1, :], st[:, 1, :nch, :])
                r4 = smallpool.tile([128, 4], F32, tag="r4")
                nc.vector.tensor_copy(r4[:, 0:2], mv[:, :, 0])
                nc.vector.tensor_mul(r4[:, 2:4], mv[:, :, 0], mv[:, :, 0])
                nc.vector.tensor_add(r4[:, 2:4], r4[:, 2:4], mv[:, :, 1])
                gs = psgn.tile([128, 4], F32, tag="gs")
                nc.tensor.matmul(gs[0:16, :], gmask_sb[:, l, :], r4[:], start=True,
                                 stop=True)
                si = smallpool.tile([128, 4], F32, tag="si")
                nc.vector.memset(si[:], 0.0)
                nc.vector.tensor_copy(si[0:16, :], gs[0:16, :])
                tmp = smallpool.tile([128, 2], F32, tag="tmp")
                nc.vector.tensor_mul(tmp[0:16, :], si[0:16, 0:2], si[0:16, 0:2])
                nc.vector.tensor_tensor(si[0:16, 2:4], si[0:16, 2:4], tmp[0:16, :],
                                        mybir.AluOpType.subtract)
                nc.scalar.activation(si[0:16, 2:4], si[0:16, 2:4], AF.Sqrt,
                                     bias=epsc[0:16, :], scale=1.0)
                nc.vector.reciprocal(si[0:16, 2:4], si[0:16, 2:4])
                bc = psgn.tile([128, 4], F32, tag="bc")
                nc.tensor.matmul(bc[:], emask_sb[:], si[:], start=True, stop=True)
                ab = abpool.tile([128, 2, 2], F32, tag="ab")
                nc.vector.tensor_mul(ab[:, :, 0], gnp_sb[:, i, t, :, 0], bc[:, 2:4])
                tmp2 = smallpool.tile([128, 2], F32, tag="tmp2")
                nc.vector.tensor_mul(tmp2[:], bc[:, 0:2], ab[:, :, 0])
                nc.vector.tensor_tensor(ab[:, :, 1], gnp_sb[:, i, t, :, 1], tmp2[:],
                                        mybir.AluOpType.subtract)
                return ab

            def get_ab(t, i, l):
                key = (t, i, l)
                if key not in ab_handles:
                    ab_handles[key] = emit_tail(t, i, l)
                return ab_handles[key]

            # ---------------- strip load + apply ----------------
            def load_strip(src_ap_r, l, s0, SR, ab):
                """DMA padded rows [s0, s0+SR+2) into an in-tile (data at
                offset 1), then optionally apply relu(a*x+b) on the interior
                (real image rows/cols only -- padded borders must stay 0)."""
                h, w = levels[l]
                wp = w + 2
                srmax = max(sum(s) for s in stripplans[l])
                smax = (srmax + 2) * wp + 3
                s_used = (SR + 2) * wp
                in_t = inpool.tile([128, 2, smax], adt, tag="instrip")
                # Single DMA covers the window plus the +-1 guard elements by
                # reading one extra border-zero element of the padded source
                # where it exists; edge strips zero the missing guards via ACT.
                src_flat = src_ap_r.rearrange("c b h w -> c b (h w)")
                lo = 1 if s0 > 0 else 0
                hi = 1 if s0 + SR < h else 0
                nc.sync.dma_start(
                    in_t[:, :, 1 - lo : 1 + s_used + hi],
                    src_flat[:, :, s0 * wp - lo : (s0 + SR + 2) * wp + hi],
                )
                if lo == 0:
                    nc.scalar.activation(
                        in_t[:, :, 0:1],
                        zta[:, 0:2].rearrange("c (b g) -> c b g", b=2),
                        AF.Identity, bias=0.0, scale=1.0,
                    )
                # tail guards (2 elems covers the odd-N matmul pad too)
                nc.scalar.activation(
                    in_t[:, :, 1 + s_used : 3 + s_used],
                    zta[:, 0:4].rearrange("c (b g) -> c b g", b=2),
                    AF.Identity, bias=0.0, scale=1.0,
                )
                if ab is not None:
                    k0 = 1 if s0 == 0 else 0
                    k1 = SR + 1 if s0 + SR == h else SR + 2
                    for cb in range(2):
                        view = (
                            in_t[:, cb, 1 + k0 * wp : 1 + k1 * wp]
                            .rearrange("c (r w) -> c r w", w=wp)[:, :, 1 : w + 1]
                        )
                        nc.scalar.activation(
                            view, view, AF.Relu,
                            bias=ab[:, cb, 1:2], scale=ab[:, cb, 0:1],
                        )
                return in_t

            # ---------------- one tower-conv job ----------------
            def conv_job(t, i, l, w_t):
                h, w = levels[l]
                wp = w + 2
                plan = _chunks(rowplans[l])
                nch = len(plan)
                src = fpad[l].ap() if i == 0 else raw[t][l][(i - 1) % 2].ap()
                src_r = src.rearrange("b c h w -> c b h w")
                dst_r = raw[t][l][i % 2].ap().rearrange("b c h w -> c b h w")
                ab = None if i == 0 else get_ab(t, i - 1, l)
                st = statspool.tile([128, 2, 34, 6], F32, tag="stats")
                stats_handles[(t, i, l)] = st
                rwmax = max(r for _, r in plan) * wp
                ci = 0
                s0 = 0
                for strip in stripplans[l]:
                    SR = sum(strip)
                    in_t = load_strip(src_r, l, s0, SR, ab)
                    r0 = s0
                    for R in strip:
                        o_base = (r0 - s0) * wp
                        n = R * wp
                        n_mm = n + (n & 1)
                        out_sb = outpool.tile(
                            [128, 2, rwmax], adt if adt != F32R else F32,
                            tag="outsb")
                        for ob in range(2):
                            ps = psmain.tile([128, 512], F32, tag="ps")
                            first = True
                            for dy in range(3):
                                for dx in range(3):
                                    for cb in range(2):
                                        o = o_base + dy * wp + dx
                                        nc.tensor.matmul(
                                            ps[:, :n_mm],
                                            w_t[:, dy, dx, cb, ob, :],
                                            in_t[:, cb, o : o + n_mm],
                                            start=first,
                                            stop=(dy == 2 and dx == 2
                                                  and cb == 1),
                                        )
                                        first = False
                            # copy interior + conv bias; zero border cols so
                            # full-width rows can be stored contiguously
                            nc.scalar.activation(
                                out_sb[:, ob, : R * wp].rearrange(
                                    "c (r w) -> c r w", w=wp
                                )[:, :, 1 : w + 1],
                                ps[:, :n].rearrange("c (r w) -> c r w", w=wp)[
                                    :, :, 1 : w + 1
                                ],
                                AF.Identity,
                                bias=cbias_sb[:, i, t, ob : ob + 1],
                                scale=1.0,
                            )
                            nc.vector.memset(
                                out_sb[:, ob, 0 : R * wp : wp], 0.0)
                            nc.vector.memset(
                                out_sb[:, ob, w + 1 : R * wp : wp], 0.0)
                            # stats over full rows incl the 2R zeros;
                            # corrected by the wp/w factor folded into gmask
                            nc.vector.bn_stats(
                                st[:, ob, ci, :], out_sb[:, ob, : R * wp])
                        for ob in range(2):
                            nc.sync.dma_start(
                                dst_r[:, ob, 1 + r0 : 1 + r0 + R, :],
                                out_sb[:, ob, : R * wp].rearrange(
                                    "c (r w) -> c r w", w=wp
                                ).bitcast(adt),
                            )
                        ci += 1
                        r0 += R
                    s0 += SR

            # ---------------- one head job ----------------
            def head_job(l):
                h, w = levels[l]
                wp = w + 2
                plan = _chunks(rowplans[l])
                ab_ct = get_ab(0, nconv - 1, l)
                ab_bt = get_ab(1, nconv - 1, l)
                ct_r = raw[0][l][(nconv - 1) % 2].ap().rearrange("b c h w -> c b h w")
                bt_r = raw[1][l][(nconv - 1) % 2].ap().rearrange("b c h w -> c b h w")
                out_ap = outs[l].ap()
                rwmax = max(r for _, r in plan) * w
                s0 = 0
                for strip in stripplans[l]:
                    SR = sum(strip)
                    in_ct = load_strip(ct_r, l, s0, SR, ab_ct)
                    in_bt = load_strip(bt_r, l, s0, SR, ab_bt)
                    r0 = s0
                    for R in strip:
                        o_base = (r0 - s0) * wp
                        n = R * wp
                        n_mm = n + (n & 1)
                        ps_ct = psmain.tile([128, 512], F32, tag="ps")
                        ps_bt = psmain.tile([128, 512], F32, tag="ps")
                        for which, (ps, in_t, w_sb, m) in enumerate(
                            [(ps_ct, in_ct, wh_ct_sb, 81),
                             (ps_bt, in_bt, wh_bt_sb, 4)]
                        ):
                            first = True
                            for dy in range(3):
                                for dx in range(3):
                                    for cb in range(2):
                                        o = o_base + dy * wp + dx
                                        nc.tensor.matmul(
                                            ps[0:m, :n_mm],
                                            w_sb[:, dy, dx, cb, :],
                                            in_t[:, cb, o : o + n_mm],
                                            start=first,
                                            stop=(dy == 2 and dx == 2
                                                  and cb == 1),
                                        )
                                        first = False
                        hd1 = hdpool.tile([128, rwmax], F32, tag="hd1")
                        hd2 = hdpool.tile([128, rwmax], F32, tag="hd2")
                        nc.scalar.activation(
                            hd1[0:81, : R * w].rearrange(
                                "c (r w) -> c r w", w=w),
                            ps_ct[0:81, :n].rearrange(
                                "c (r w) -> c r w", w=wp)[:, :, 1 : w + 1],
                            AF.Identity,
                            bias=hb_sb[0:81, :],
                            scale=1.0,
                        )
                        nc.scalar.activation(
                            hd2[0:4, : R * w].rearrange(
                                "c (r w) -> c r w", w=w),
                            ps_bt[0:4, :n].rearrange(
                                "c (r w) -> c r w", w=wp)[:, :, 1 : w + 1],
                            AF.Exp,
                            bias=hexp_b_sb[0:4, l : l + 1],
                            scale=hexp_s_sb[0:4, l : l + 1],
                        )
                        nc.sync.dma_start(
                            out_ap[0:80, r0 : r0 + R, :], hd1[0:80, : R * w]
                        )
                        nc.sync.dma_start(
                            out_ap[84:85, r0 : r0 + R, :], hd1[80:81, : R * w]
                        )
                        nc.sync.dma_start(
                            out_ap[80:84, r0 : r0 + R, :], hd2[0:4, : R * w]
                        )
                        r0 += R
                    s0 += SR

            # ---------------- emission: layer-major waves ----------------
            def emit_all():
                for i in range(nconv):
                    for t in range(2):
                        w_t = wpool.tile([128, 3, 3, 2, 2, 128], adt, tag="wt")
                        nc.sync.dma_start(w_t[:], wts[:, i, t])
                        for l in range(nlev):
                            conv_job(t, i, l, w_t)
                for l in range(nlev):
                    head_job(l)

            if nrep == 1:
                emit_all()
            else:
                with tc.For_i(0, nrep, 1):
                    emit_all()

    nc.compile()
    return nc


# ---------------- host side ----------------

_NC_CACHE = {}


def _get_nc():
    if "nc" not in _NC_CACHE:
        _NC_CACHE["nc"] = build_nc()
    return _NC_CACHE["nc"]


def _prep_common(cls_w, cls_b, cls_gn_g, cls_gn_b, box_w, box_b, box_gn_g,
                 box_gn_b, logits_w, logits_b, ctr_w, ctr_b, reg_w, reg_b,
                 scales, nlev, levels=None):
    levels = levels or LEVELS
    f32 = np.float32
    # tower weights: [ci, conv, T, dy, dx, cb, ob, co]
    wt = np.stack([cls_w, box_w], axis=1).astype(f32)  # [4, 2, 256o, 256i, 3, 3]
    wt = wt.reshape(NCONV, 2, 2, 128, 2, 128, 3, 3)  # [i,T,ob,co,cb,ci,dy,dx]
    wts = np.ascontiguousarray(wt.transpose(5, 0, 1, 6, 7, 4, 2, 3))
    # head weights
    hw_ct = np.concatenate([logits_w, ctr_w], axis=0).astype(f32)  # [81,256,3,3]
    hw_ct = hw_ct.reshape(81, 2, 128, 3, 3)
    wh_ct = np.ascontiguousarray(hw_ct.transpose(2, 3, 4, 1, 0))  # [ci,dy,dx,cb,81]
    hw_bt = reg_w.astype(f32).reshape(4, 2, 128, 3, 3)
    wh_bt = np.ascontiguousarray(hw_bt.transpose(2, 3, 4, 1, 0))
    # GN params [p, conv, T, blk, {g,b}]
    gg = np.stack([cls_gn_g, box_gn_g], axis=1).reshape(NCONV, 2, 2, 128)
    gb = np.stack([cls_gn_b, box_gn_b], axis=1).reshape(NCONV, 2, 2, 128)
    gnp = np.ascontiguousarray(
        np.stack([gg, gb], axis=-1).transpose(3, 0, 1, 2, 4)
    ).astype(f32)
    cb_ = np.stack([cls_b, box_b], axis=1).reshape(NCONV, 2, 2, 128)
    cbias = np.ascontiguousarray(cb_.transpose(3, 0, 1, 2)).astype(f32)
    hb = np.zeros((128, 1), f32)
    hb[0:80, 0] = logits_b
    hb[80, 0] = ctr_b[0]
    hexp_s = np.zeros((128, nlev), f32)
    hexp_b = np.zeros((128, nlev), f32)
    for l in range(nlev):
        hexp_s[0:4, l] = scales[l]
        hexp_b[0:4, l] = scales[l] * reg_b
    gmask = np.zeros((128, nlev, 16), f32)
    for l, (h_, w_) in enumerate(levels):
        fcorr = (w_ + 2.0) / w_
        for p in range(128):
            gmask[p, l, p // 8] = fcorr / 8.0
    emask = np.zeros((128, 128), f32)
    for c_ in range(128):
        emask[c_ // 8, c_] = 1.0
    anp = mybir.dt.np(ADT)
    return dict(wts=wts.astype(anp), wh_ct=wh_ct.astype(anp),
                wh_bt=wh_bt.astype(anp), gnp=gnp, cbias=cbias,
                hb=hb, hexp_s=hexp_s, hexp_b=hexp_b, gmask=gmask, emask=emask)


def _locations(levels, strides):
    locs = []
    for l, (h, w) in enumerate(levels):
        s = strides[l]
        sx = np.arange(w, dtype=np.float32) * s + s // 2
        sy = np.arange(h, dtype=np.float32) * s + s // 2
        yy, xx = np.meshgrid(sy, sx, indexing="ij")
        locs.append(np.stack([xx.reshape(-1), yy.reshape(-1)], axis=1))
    return np.concatenate(locs, axis=0).astype(np.float32)


def _make_in_maps(inputs):
    feats = [np.asarray(inputs[f"feat{l}"], np.float32) for l in range(len(LEVELS))]
    B = feats[0].shape[0]
    assert B == N_CORES
    nlev = len(LEVELS)
    common = _prep_common(
        np.asarray(inputs["cls_w"]), np.asarray(inputs["cls_b"]),
        np.asarray(inputs["cls_gn_g"]), np.asarray(inputs["cls_gn_b"]),
        np.asarray(inputs["box_w"]), np.asarray(inputs["box_b"]),
        np.asarray(inputs["box_gn_g"]), np.asarray(inputs["box_gn_b"]),
        np.asarray(inputs["logits_w"]), np.asarray(inputs["logits_b"]),
        np.asarray(inputs["ctr_w"]), np.asarray(inputs["ctr_b"]),
        np.asarray(inputs["reg_w"]), np.asarray(inputs["reg_b"]),
        np.asarray(inputs["scales"]), nlev)
    in_maps = []
    for b in range(B):
        m = dict(common)
        for l, (h, w) in enumerate(LEVELS):
            f = feats[l][b].reshape(2, 128, h, w)
            m[f"fpad{l}"] = np.pad(
                f, ((0, 0), (0, 0), (1, 1), (1, 1))
            ).astype(mybir.dt.np(ADT))
        in_maps.append(m)
    return in_maps


def kernel(feat0, feat1, feat2, feat3, feat4, cls_w, cls_b, cls_gn_g, cls_gn_b,
           box_w, box_b, box_gn_g, box_gn_b, logits_w, logits_b, ctr_w, ctr_b,
           reg_w, reg_b, scales):
    in_maps = _make_in_maps(dict(
        feat0=feat0, feat1=feat1, feat2=feat2, feat3=feat3, feat4=feat4,
        cls_w=cls_w, cls_b=cls_b, cls_gn_g=cls_gn_g, cls_gn_b=cls_gn_b,
        box_w=box_w, box_b=box_b, box_gn_g=box_gn_g, box_gn_b=box_gn_b,
        logits_w=logits_w, logits_b=logits_b, ctr_w=ctr_w, ctr_b=ctr_b,
        reg_w=reg_w, reg_b=reg_b, scales=scales))
    B = N_CORES
    nlev = len(LEVELS)

    nc = _get_nc()
    res = run_bass_kernel_spmd(nc, in_maps, list(range(N_CORES)))

    out_rows = []
    for b in range(B):
        parts = [res.results[b][f"out{l}"].reshape(-1) for l in range(nlev)]
        out_rows.append(np.concatenate(parts))
    out = np.stack(out_rows).astype(np.float32)
    locs = _locations(LEVELS, STRIDES)
    return out, locs


if __name__ == "__main__":
    import time

    t0 = time.time()
    nc = build_nc()
    print(f"build_nc: {time.time() - t0:.1f}s")
